# revision 25
# baseline (speedup 1.0000x reference)
"""Single-head attention (N=8192, EMB=QDIM=KDIM=VDIM=1024, fp32) on 8 TRN2
NeuronCores.

Strategy (sequence-parallel / ring-attention style):
  - Q rows sharded 1024/core. Each core computes its Q^T, K^T, V shards from
    its x shard (weights replicated), all-gathers K^T and V across the 8
    cores via chunked collectives (4 chunks each, overlapped with compute),
    then runs flash-style attention of its 1024 Q rows against all 8192 keys.
  - All matmuls run in float32r (full PE rate, ~1e-4 rel err). The f32->f32r
    rounding rides the PSUM-evacuation activation copies.
  - Layout is fully transposed so no on-device transposes are needed:
      Q^T[dq,q]   = Wq @ x^T   (lhsT=WqT chunk, rhs=xT chunk)    + bq via ACT bias
      K^T[dk,k]   = Wk @ x^T                                      + bk via ACT bias
      V[k,dv]     = x @ Wv^T   (lhsT=xT chunk, rhs=WvT chunk)     (bv folded at end)
      S^T[k,q]    = K^T.T-chunks @ Q^T  (lhsT=K^T tile, rhs=Q^T tile)
      P^T[k,q]    = exp(S^T / 32)       (ACT, no max-subtraction: logits in [-2,2])
      O^T[dv,q]  += V-chunk.T @ P^T     (lhsT=V tile, rhs=P^T tile)
      D[128,q]   += ones128.T @ P^T     (softmax denominators, every row = D)
      out^T       = O^T * (1/D) + bv    (DVE)
  - Attention slabs are 512 keys pairing two ranks' 256-key gather chunks,
    so the first slabs are ready after just one (K,V) collective pair.
  - Final output assembled on host: concat of per-core out^T.T.
"""
import sys
import types

import numpy as np

sys.path.insert(0, "/opt/trn_rl_repo")

import concourse.bass as bass
import concourse.bass_isa as bass_isa
import concourse.mybir as mybir
import concourse.tile as tile
from concourse import bacc
from concourse.bass_utils import run_bass_kernel_spmd

# ---- register the NTFF profiling hook missing from the slim antenv stub ----
def _ensure_profile_hook():
    try:
        import antenv
        if "antenv.axon_hooks" in sys.modules:
            return
        import trn_agent_boot.trn_boot as trn_boot
        hook = trn_boot._ntff_profile_via_ctypes("/opt/axon/libaxon_pjrt.so")
        mod = types.ModuleType("antenv.axon_hooks")
        mod.get_axon_ntff_profile_hook = lambda: hook
        mod.set_axon_ntff_profile_hook = lambda h: None
        sys.modules["antenv.axon_hooks"] = mod
        antenv.axon_hooks = mod
    except Exception:
        pass


N_CORES = 8
EMB = 1024
N = 8192
NQ = N // N_CORES          # 1024 q rows per core
NCH = EMB // 128           # 8 partition chunks of the 1024-dim axes
G = 4                      # all-gather chunks per K / per V
KCH = NQ // G              # 256 keys per gather chunk per rank
QT_TILES = 2
QTW = NQ // QT_TILES       # 512
SCALE = 1.0 / 32.0         # 1/sqrt(KDIM)

F32 = mybir.dt.float32
F32R = mybir.dt.float32r
BF16 = mybir.dt.bfloat16
ATT = BF16      # dtype of attention-stage operands (Q^T/K^T/V/P^T, gathers)

_COMPILED = {}


def _build(emb=EMB, n=N):
    global EMB, N, NQ, NCH, KCH, QTW
    EMB, N = emb, n
    NQ = N // N_CORES
    NCH = EMB // 128
    KCH = NQ // G
    QTW = NQ // QT_TILES
    SLAB_KC = 2 * KCH // 128   # 128-key chunks per attention slab (2 ranks)
    VKC = KCH // 128           # 128-key chunks per gather chunk
    VW = min(512, EMB)         # dv tile width for the V projection
    DVN = EMB // VW
    nc = bacc.Bacc("TRN2", target_bir_lowering=False, debug=False,
                   num_devices=N_CORES)

    # x/W inputs are declared f32r: same 4-byte values, consumed by the PE
    # directly, so no on-device rounding pass is needed
    xT = nc.dram_tensor("xT", [EMB, NQ], F32R, kind="ExternalInput")
    wqT = nc.dram_tensor("wqT", [EMB, EMB], F32R, kind="ExternalInput")
    wkT = nc.dram_tensor("wkT", [EMB, EMB], F32R, kind="ExternalInput")
    wvT = nc.dram_tensor("wvT", [EMB, EMB], F32R, kind="ExternalInput")
    bq = nc.dram_tensor("bq", [EMB], F32, kind="ExternalInput")
    bk = nc.dram_tensor("bk", [EMB], F32, kind="ExternalInput")
    bv = nc.dram_tensor("bv", [EMB], F32, kind="ExternalInput")
    outT = nc.dram_tensor("outT", [EMB, NQ], F32, kind="ExternalOutput")

    k_bounce = [nc.dram_tensor(f"k_bounce{c}", [EMB, KCH], F32R)
                for c in range(G)]
    v_bounce = [nc.dram_tensor(f"v_bounce{c}", [KCH, EMB], ATT)
                for c in range(G)]
    k_gath = [nc.dram_tensor(f"k_gath{c}", [N_CORES, EMB, KCH], F32R,
                             addr_space="Shared") for c in range(G)]
    v_gath = [nc.dram_tensor(f"v_gath{c}", [N_CORES, KCH, EMB], ATT,
                             addr_space="Shared") for c in range(G)]
    rg = [list(range(N_CORES))]

    def ag(which, c):
        b, g_ = (k_bounce, k_gath) if which == "k" else (v_bounce, v_gath)
        nc.gpsimd.collective_compute(
            "AllGather", mybir.AluOpType.bypass, replica_groups=rg,
            ins=[b[c][:]], outs=[g_[c][:]])

    with tile.TileContext(nc) as tc:
        with (
            tc.tile_pool(name="bias", bufs=1) as bias_p,
            tc.tile_pool(name="qt", bufs=1) as qt_p,
            tc.tile_pool(name="oacc", bufs=1) as oacc_p,
            # ev outlives the projection scope: its tiles are read by DMA
            # (bounce stores); letting another pool recycle the slots trips
            # a Tile DMA-queue under-sync (sim-verified race)
            tc.tile_pool(name="ev", bufs=6) as ev_p,
            tc.tile_pool(name="pf", bufs=1) as pf_p,
        ):
            bq_t = [bias_p.tile([128, 1], F32, name=f"bq{m}") for m in range(NCH)]
            bk_t = [bias_p.tile([128, 1], F32, name=f"bk{m}") for m in range(NCH)]
            bv_t = [bias_p.tile([128, 1], F32, name=f"bv{m}") for m in range(NCH)]
            ones_kr = bias_p.tile([128, 128], ATT, name="ones_kr")
            ones_f32 = bias_p.tile([128, 128], F32, name="ones_f32")
            nc.vector.memset(ones_f32[:], 1.0)
            nc.vector.tensor_copy(ones_kr[:], ones_f32[:])

            qt = [qt_p.tile([128, NQ], F32R, name=f"qt{m}") for m in range(NCH)]
            o_acc = [[oacc_p.tile([128, QTW], F32, name=f"oa{q}_{m}")
                      for m in range(NCH)] for q in range(QT_TILES)]

            # ================= projection phase =================
            with (
                tc.tile_pool(name="wr", bufs=1) as wr_p,
                tc.tile_pool(name="xr", bufs=1) as xr_p,
                tc.tile_pool(name="psP", bufs=1, space="PSUM") as psP_p,
            ):
                # load xT and Wk interleaved (Wk needed first), direct f32r
                xr = [xr_p.tile([128, NQ], F32R, name=f"xr{k}")
                      for k in range(NCH)]
                wk_t = [wr_p.tile([128, EMB], F32R, name=f"awk{k}", tag=f"a{k}")
                        for k in range(NCH)]
                for k in range(NCH):
                    nc.sync.dma_start(xr[k][:], xT[bass.ts(k, 128), :])
                    nc.sync.dma_start(wk_t[k][:], wkT[bass.ts(k, 128), :])

                def load_w(wT, tagpfx):
                    wt = [wr_p.tile([128, EMB], F32R, name=f"{tagpfx}w{k}",
                                    tag=f"{tagpfx}{k}") for k in range(NCH)]
                    for k in range(NCH):
                        nc.sync.dma_start(wt[k][:], wT[bass.ts(k, 128), :])
                    return wt

                wv_t = load_w(wvT, "b")
                # bias loads ride the idle GpSimd DMA queue, off the
                # sync-queue critical path of the x/W loads
                for m in range(NCH):
                    sl = bass.ts(m, 128)
                    nc.gpsimd.dma_start(bq_t[m][:], bq[sl].unsqueeze(1))
                    nc.gpsimd.dma_start(bk_t[m][:], bk[sl].unsqueeze(1))
                    nc.gpsimd.dma_start(bv_t[m][:], bv[sl].unsqueeze(1))

                def k_proj_cc(cc):
                    # K^T columns -> k_bounce[2cc], [2cc+1]; k-outer over 8
                    # live PSUM banks so the PE starts on first-arrived chunks
                    pss = [psP_p.tile([128, 2 * KCH], F32, name=f"psk{m}",
                                      tag=f"pp{m}") for m in range(NCH)]
                    for k in range(NCH):
                        for m in range(NCH):
                            nc.tensor.matmul(pss[m][:],
                                             wk_t[k][:, bass.ts(m, 128)],
                                             xr[k][:, bass.ts(cc, 2 * KCH)],
                                             start=(k == 0), stop=(k == NCH - 1))
                    for m in range(NCH):
                        ev = ev_p.tile([128, 2 * KCH], F32R, name="evk",
                                       tag="ev")
                        nc.scalar.activation(ev[:], pss[m][:],
                                             mybir.ActivationFunctionType.Identity,
                                             bias=bk_t[m][:])
                        nc.sync.dma_start(k_bounce[2 * cc][bass.ts(m, 128), :],
                                          ev[:, :KCH])
                        nc.sync.dma_start(
                            k_bounce[2 * cc + 1][bass.ts(m, 128), :],
                            ev[:, KCH:])

                def v_proj_cc(cc):
                    # V rows -> v_bounce[2cc], [2cc+1]; k-outer
                    groups = [(cc * 2 * VKC + mm, nn)
                              for mm in range(2 * VKC) for nn in range(DVN)]
                    pss = [psP_p.tile([128, VW], F32, name=f"psv{g}",
                                      tag=f"pp{g}") for g in range(len(groups))]
                    for k in range(NCH):
                        for g, (m, nn) in enumerate(groups):
                            nc.tensor.matmul(
                                pss[g][:], xr[k][:, bass.ts(m, 128)],
                                wv_t[k][:, bass.ts(nn, VW)],
                                start=(k == 0), stop=(k == NCH - 1))
                    for g, (m, nn) in enumerate(groups):
                        ev = ev_p.tile([128, VW], ATT, name="evv", tag="ev")
                        nc.scalar.activation(
                            ev[:], pss[g][:],
                            mybir.ActivationFunctionType.Copy)
                        nc.sync.dma_start(
                            v_bounce[m // VKC][bass.ts(m % VKC, 128),
                                               bass.ts(nn, VW)], ev[:])

                k_proj_cc(0)
                v_proj_cc(0)
                ag("k", 0)
                ag("v", 0)
                # prefetch slab 0 (chunk 0, ranks 0+1) into outer tiles so
                # attention starts the moment the projections drain
                kt_pf = pf_p.tile([128, NCH * 2 * KCH], F32R, name="ktpf")
                for j in range(2):
                    for k in range(NCH):
                        nc.sync.dma_start(
                            kt_pf[:, bass.ds(k * 2 * KCH + j * KCH, KCH)],
                            k_gath[0][j][bass.ts(k, 128), :])
                v_pf = pf_p.tile([128, SLAB_KC * EMB], ATT, name="vpf")
                for j in range(2):
                    for kc in range(VKC):
                        nc.sync.dma_start(
                            v_pf[:, bass.ds((j * VKC + kc) * EMB, EMB)],
                            v_gath[0][j][bass.ts(kc, 128), :])
                k_proj_cc(1)
                v_proj_cc(1)

                wq_t = load_w(wqT, "a")  # reuse Wk slots
                for nn in range(QT_TILES):
                    pss = [psP_p.tile([128, QTW], F32, name=f"psq{m}",
                                      tag=f"pp{m}") for m in range(NCH)]
                    for k in range(NCH):
                        for m in range(NCH):
                            nc.tensor.matmul(pss[m][:],
                                             wq_t[k][:, bass.ts(m, 128)],
                                             xr[k][:, bass.ts(nn, QTW)],
                                             start=(k == 0), stop=(k == NCH - 1))
                    for m in range(NCH):
                        nc.scalar.activation(qt[m][:, bass.ts(nn, QTW)],
                                             pss[m][:],
                                             mybir.ActivationFunctionType.Identity,
                                             bias=bq_t[m][:])

            # Fence the proj->attention pool-slot handoff (sim-verified
            # DMA-queue under-sync on recycled slots). Remaining gathers are
            # issued after the fence so it does not wait on them.
            tc.strict_bb_all_engine_barrier()
            ag("k", 1)
            ag("v", 1)
            ag("k", 2)
            ag("v", 2)
            ag("k", 3)
            ag("v", 3)

            # ================= attention phase =================
            with (
                tc.tile_pool(name="kv", bufs=3) as kv_p,
                tc.tile_pool(name="pt", bufs=8) as pt_p,
                tc.tile_pool(name="fin", bufs=2) as fin_p,
                tc.tile_pool(name="psS", bufs=3, space="PSUM") as psS_p,
                tc.tile_pool(name="psO", bufs=2, space="PSUM") as psO_p,
                tc.tile_pool(name="psD", bufs=1, space="PSUM") as psD_p,
            ):
                d_ps = [psD_p.tile([128, QTW], F32, name=f"dps{q}",
                                   tag=f"d{q}") for q in range(QT_TILES)]

                # slabs: 512 keys = gather chunk c of ranks (2p, 2p+1)
                slabs = [(c, p) for c in range(G) for p in range(N_CORES // 2)]
                for si, (c, p) in enumerate(slabs):
                    first, last = si == 0, si == len(slabs) - 1
                    if first:
                        kt_slab, v_slab = kt_pf, v_pf
                    else:
                        kt_slab = kv_p.tile([128, NCH * 2 * KCH], F32R,
                                            name="kts", tag="kt")
                        for j in range(2):
                            for k in range(NCH):
                                nc.sync.dma_start(
                                    kt_slab[:, bass.ds(k * 2 * KCH + j * KCH,
                                                       KCH)],
                                    k_gath[c][2 * p + j][bass.ts(k, 128), :])
                        v_slab = kv_p.tile([128, SLAB_KC * EMB], ATT,
                                           name="vs", tag="vt")
                        for j in range(2):
                            for kc in range(VKC):
                                nc.sync.dma_start(
                                    v_slab[:, bass.ds((j * VKC + kc) * EMB,
                                                      EMB)],
                                    v_gath[c][2 * p + j]
                                    [bass.ts(kc, 128), :])

                    for q in range(QT_TILES):
                        pts = []
                        for kc in range(SLAB_KC):
                            ps = psS_p.tile([128, QTW], F32, name="pss",
                                            tag="ps")
                            for k in range(NCH):
                                nc.tensor.matmul(
                                    ps[:],
                                    kt_slab[:, bass.ds(k * 2 * KCH + kc * 128,
                                                       128)],
                                    qt[k][:, bass.ts(q, QTW)],
                                    start=(k == 0), stop=(k == NCH - 1))
                            pt = pt_p.tile([128, QTW], ATT, name="pt",
                                           tag="pt")
                            nc.scalar.activation(
                                pt[:], ps[:],
                                mybir.ActivationFunctionType.Exp, scale=SCALE)
                            pts.append(pt)
                        # presum the slab's P^T tiles on DVE (bf16 4x) so
                        # the denominator costs one PE matmul per slab
                        pp = pt_p.tile([128, QTW], ATT, name="pp", tag="pp",
                                       bufs=3)
                        nc.vector.tensor_add(pp[:], pts[0][:], pts[1][:])
                        for kc in range(2, SLAB_KC):
                            nc.vector.tensor_add(pp[:], pp[:], pts[kc][:])
                        nc.tensor.matmul(d_ps[q][:], ones_kr[:], pp[:],
                                         start=first, stop=last)
                        for m in range(NCH):
                            po = psO_p.tile([128, QTW], F32, name="pso",
                                            tag="po")
                            for kc in range(SLAB_KC):
                                nc.tensor.matmul(
                                    po[:],
                                    v_slab[:, bass.ds(kc * EMB + m * 128, 128)],
                                    pts[kc][:],
                                    start=(kc == 0), stop=(kc == SLAB_KC - 1))
                            if first:
                                nc.vector.tensor_copy(o_acc[q][m][:], po[:])
                            else:
                                nc.vector.tensor_add(o_acc[q][m][:],
                                                     o_acc[q][m][:], po[:])

                # ---- finalize: out^T = O^T * (1/D) + bv ----
                for q in range(QT_TILES):
                    bc = fin_p.tile([128, QTW], F32, name="bc", tag="bcs")
                    nc.vector.reciprocal(bc[:], d_ps[q][:])
                    for m in range(NCH):
                        fo = fin_p.tile([128, QTW], F32, name="fo", tag="fo",
                                        bufs=4)
                        nc.vector.tensor_mul(fo[:], o_acc[q][m][:], bc[:])
                        nc.vector.tensor_scalar_add(fo[:], fo[:], bv_t[m][:])
                        nc.sync.dma_start(
                            outT[bass.ts(m, 128), bass.ts(q, QTW)], fo[:])

    nc.compile()
    return nc


def _get_nc():
    if "nc" not in _COMPILED:
        _COMPILED["nc"] = _build()
    return _COMPILED["nc"]


def kernel(x, Wq, bq, Wk, bk, Wv, bv, _trace=False):
    _ensure_profile_hook()
    nc = _get_nc()
    x = np.ascontiguousarray(np.asarray(x, dtype=np.float32))
    in_maps = []
    wqT = np.ascontiguousarray(np.asarray(Wq, np.float32).T)
    wkT = np.ascontiguousarray(np.asarray(Wk, np.float32).T)
    wvT = np.ascontiguousarray(np.asarray(Wv, np.float32).T)
    bq = np.ascontiguousarray(np.asarray(bq, np.float32))
    bk = np.ascontiguousarray(np.asarray(bk, np.float32))
    bv = np.ascontiguousarray(np.asarray(bv, np.float32))
    for i in range(N_CORES):
        xT_i = np.ascontiguousarray(x[i * NQ:(i + 1) * NQ].T)
        in_maps.append({"xT": xT_i, "wqT": wqT, "wkT": wkT, "wvT": wvT,
                        "bq": bq, "bk": bk, "bv": bv})
    res = run_bass_kernel_spmd(nc, in_maps, core_ids=list(range(N_CORES)),
                               trace=_trace)
    out = np.concatenate([res.results[i]["outT"].T for i in range(N_CORES)],
                         axis=0)
    if _trace:
        return out, res
    return out


# revision 26
# speedup vs baseline: 1.0175x; 1.0175x over previous
"""Single-head attention (N=8192, EMB=QDIM=KDIM=VDIM=1024, fp32) on 8 TRN2
NeuronCores.

Strategy (sequence-parallel / ring-attention style):
  - Q rows sharded 1024/core. Each core computes its Q^T, K^T, V shards from
    its x shard (weights replicated), all-gathers K^T and V across the 8
    cores via chunked collectives (4 chunks each, overlapped with compute),
    then runs flash-style attention of its 1024 Q rows against all 8192 keys.
  - All matmuls run in float32r (full PE rate, ~1e-4 rel err). The f32->f32r
    rounding rides the PSUM-evacuation activation copies.
  - Layout is fully transposed so no on-device transposes are needed:
      Q^T[dq,q]   = Wq @ x^T   (lhsT=WqT chunk, rhs=xT chunk)    + bq via ACT bias
      K^T[dk,k]   = Wk @ x^T                                      + bk via ACT bias
      V[k,dv]     = x @ Wv^T   (lhsT=xT chunk, rhs=WvT chunk)     (bv folded at end)
      S^T[k,q]    = K^T.T-chunks @ Q^T  (lhsT=K^T tile, rhs=Q^T tile)
      P^T[k,q]    = exp(S^T / 32)       (ACT, no max-subtraction: logits in [-2,2])
      O^T[dv,q]  += V-chunk.T @ P^T     (lhsT=V tile, rhs=P^T tile)
      D[128,q]   += ones128.T @ P^T     (softmax denominators, every row = D)
      out^T       = O^T * (1/D) + bv    (DVE)
  - Attention slabs are 512 keys pairing two ranks' 256-key gather chunks,
    so the first slabs are ready after just one (K,V) collective pair.
  - Final output assembled on host: concat of per-core out^T.T.
"""
import sys
import types

import numpy as np

sys.path.insert(0, "/opt/trn_rl_repo")

import concourse.bass as bass
import concourse.bass_isa as bass_isa
import concourse.mybir as mybir
import concourse.tile as tile
from concourse import bacc
from concourse.bass_utils import run_bass_kernel_spmd

# ---- register the NTFF profiling hook missing from the slim antenv stub ----
def _ensure_profile_hook():
    try:
        import antenv
        if "antenv.axon_hooks" in sys.modules:
            return
        import trn_agent_boot.trn_boot as trn_boot
        hook = trn_boot._ntff_profile_via_ctypes("/opt/axon/libaxon_pjrt.so")
        mod = types.ModuleType("antenv.axon_hooks")
        mod.get_axon_ntff_profile_hook = lambda: hook
        mod.set_axon_ntff_profile_hook = lambda h: None
        sys.modules["antenv.axon_hooks"] = mod
        antenv.axon_hooks = mod
    except Exception:
        pass


N_CORES = 8
EMB = 1024
N = 8192
NQ = N // N_CORES          # 1024 q rows per core
NCH = EMB // 128           # 8 partition chunks of the 1024-dim axes
G = 4                      # all-gather chunks per K / per V
KCH = NQ // G              # 256 keys per gather chunk per rank
QT_TILES = 2
QTW = NQ // QT_TILES       # 512
SCALE = 1.0 / 32.0         # 1/sqrt(KDIM)

F32 = mybir.dt.float32
F32R = mybir.dt.float32r
BF16 = mybir.dt.bfloat16
ATT = BF16      # dtype of attention-stage operands (Q^T/K^T/V/P^T, gathers)

_COMPILED = {}


def _build(emb=EMB, n=N):
    global EMB, N, NQ, NCH, KCH, QTW
    EMB, N = emb, n
    NQ = N // N_CORES
    NCH = EMB // 128
    KCH = NQ // G
    QTW = NQ // QT_TILES
    SLAB_KC = 2 * KCH // 128   # 128-key chunks per attention slab (2 ranks)
    VKC = KCH // 128           # 128-key chunks per gather chunk
    VW = min(512, EMB)         # dv tile width for the V projection
    DVN = EMB // VW
    nc = bacc.Bacc("TRN2", target_bir_lowering=False, debug=False,
                   num_devices=N_CORES)

    # x/W inputs are declared f32r: same 4-byte values, consumed by the PE
    # directly, so no on-device rounding pass is needed
    xT = nc.dram_tensor("xT", [EMB, NQ], F32R, kind="ExternalInput")
    wqT = nc.dram_tensor("wqT", [EMB, EMB], F32R, kind="ExternalInput")
    wkT = nc.dram_tensor("wkT", [EMB, EMB], F32R, kind="ExternalInput")
    wvT = nc.dram_tensor("wvT", [EMB, EMB], F32R, kind="ExternalInput")
    bq = nc.dram_tensor("bq", [EMB], F32, kind="ExternalInput")
    bk = nc.dram_tensor("bk", [EMB], F32, kind="ExternalInput")
    bv = nc.dram_tensor("bv", [EMB], F32, kind="ExternalInput")
    outT = nc.dram_tensor("outT", [EMB, NQ], F32, kind="ExternalOutput")

    k_bounce = [nc.dram_tensor(f"k_bounce{c}", [EMB, KCH], F32R)
                for c in range(G)]
    v_bounce = [nc.dram_tensor(f"v_bounce{c}", [KCH, EMB], ATT)
                for c in range(G)]
    k_gath = [nc.dram_tensor(f"k_gath{c}", [N_CORES, EMB, KCH], F32R,
                             addr_space="Shared") for c in range(G)]
    v_gath = [nc.dram_tensor(f"v_gath{c}", [N_CORES, KCH, EMB], ATT,
                             addr_space="Shared") for c in range(G)]
    rg = [list(range(N_CORES))]

    def ag(which, c):
        b, g_ = (k_bounce, k_gath) if which == "k" else (v_bounce, v_gath)
        nc.gpsimd.collective_compute(
            "AllGather", mybir.AluOpType.bypass, replica_groups=rg,
            ins=[b[c][:]], outs=[g_[c][:]])

    with tile.TileContext(nc) as tc:
        with (
            tc.tile_pool(name="bias", bufs=1) as bias_p,
            tc.tile_pool(name="qt", bufs=1) as qt_p,
            tc.tile_pool(name="oacc", bufs=1) as oacc_p,
            # ev outlives the projection scope: its tiles are read by DMA
            # (bounce stores); letting another pool recycle the slots trips
            # a Tile DMA-queue under-sync (sim-verified race)
            tc.tile_pool(name="ev", bufs=6) as ev_p,
            tc.tile_pool(name="pf", bufs=1) as pf_p,
        ):
            bq_t = [bias_p.tile([128, 1], F32, name=f"bq{m}") for m in range(NCH)]
            bk_t = [bias_p.tile([128, 1], F32, name=f"bk{m}") for m in range(NCH)]
            bv_t = [bias_p.tile([128, 1], F32, name=f"bv{m}") for m in range(NCH)]
            ones_kr = bias_p.tile([128, 128], ATT, name="ones_kr")
            ones_f32 = bias_p.tile([128, 128], F32, name="ones_f32")
            nc.vector.memset(ones_f32[:], 1.0)
            nc.vector.tensor_copy(ones_kr[:], ones_f32[:])

            qt = [qt_p.tile([128, NQ], F32R, name=f"qt{m}") for m in range(NCH)]
            o_acc = [[oacc_p.tile([128, QTW], F32, name=f"oa{q}_{m}")
                      for m in range(NCH)] for q in range(QT_TILES)]

            # ================= projection phase =================
            with (
                tc.tile_pool(name="wr", bufs=1) as wr_p,
                tc.tile_pool(name="xr", bufs=1) as xr_p,
                tc.tile_pool(name="psP", bufs=1, space="PSUM") as psP_p,
            ):
                # load xT and Wk interleaved (Wk needed first), direct f32r
                xr = [xr_p.tile([128, NQ], F32R, name=f"xr{k}")
                      for k in range(NCH)]
                wk_t = [wr_p.tile([128, EMB], F32R, name=f"awk{k}", tag=f"a{k}")
                        for k in range(NCH)]
                for k in range(NCH):
                    nc.sync.dma_start(xr[k][:], xT[bass.ts(k, 128), :])
                    nc.sync.dma_start(wk_t[k][:], wkT[bass.ts(k, 128), :])

                def load_w(wT, tagpfx):
                    wt = [wr_p.tile([128, EMB], F32R, name=f"{tagpfx}w{k}",
                                    tag=f"{tagpfx}{k}") for k in range(NCH)]
                    for k in range(NCH):
                        nc.sync.dma_start(wt[k][:], wT[bass.ts(k, 128), :])
                    return wt

                wv_t = load_w(wvT, "b")
                # bias loads ride the idle GpSimd DMA queue, off the
                # sync-queue critical path of the x/W loads
                for m in range(NCH):
                    sl = bass.ts(m, 128)
                    nc.gpsimd.dma_start(bq_t[m][:], bq[sl].unsqueeze(1))
                    nc.gpsimd.dma_start(bk_t[m][:], bk[sl].unsqueeze(1))
                    nc.gpsimd.dma_start(bv_t[m][:], bv[sl].unsqueeze(1))

                def k_proj_cc(cc):
                    # K^T columns -> k_bounce[2cc], [2cc+1]; k-outer over 8
                    # live PSUM banks so the PE starts on first-arrived chunks
                    pss = [psP_p.tile([128, 2 * KCH], F32, name=f"psk{m}",
                                      tag=f"pp{m}") for m in range(NCH)]
                    for k in range(NCH):
                        for m in range(NCH):
                            nc.tensor.matmul(pss[m][:],
                                             wk_t[k][:, bass.ts(m, 128)],
                                             xr[k][:, bass.ts(cc, 2 * KCH)],
                                             start=(k == 0), stop=(k == NCH - 1))
                    for m in range(NCH):
                        ev = ev_p.tile([128, 2 * KCH], F32R, name="evk",
                                       tag="ev")
                        nc.scalar.activation(ev[:], pss[m][:],
                                             mybir.ActivationFunctionType.Identity,
                                             bias=bk_t[m][:])
                        nc.sync.dma_start(k_bounce[2 * cc][bass.ts(m, 128), :],
                                          ev[:, :KCH])
                        nc.sync.dma_start(
                            k_bounce[2 * cc + 1][bass.ts(m, 128), :],
                            ev[:, KCH:])

                def v_proj_cc(cc):
                    # V rows -> v_bounce[2cc], [2cc+1]; k-outer
                    groups = [(cc * 2 * VKC + mm, nn)
                              for mm in range(2 * VKC) for nn in range(DVN)]
                    pss = [psP_p.tile([128, VW], F32, name=f"psv{g}",
                                      tag=f"pp{g}") for g in range(len(groups))]
                    for k in range(NCH):
                        for g, (m, nn) in enumerate(groups):
                            nc.tensor.matmul(
                                pss[g][:], xr[k][:, bass.ts(m, 128)],
                                wv_t[k][:, bass.ts(nn, VW)],
                                start=(k == 0), stop=(k == NCH - 1))
                    for g, (m, nn) in enumerate(groups):
                        ev = ev_p.tile([128, VW], ATT, name="evv", tag="ev")
                        nc.scalar.activation(
                            ev[:], pss[g][:],
                            mybir.ActivationFunctionType.Copy)
                        nc.sync.dma_start(
                            v_bounce[m // VKC][bass.ts(m % VKC, 128),
                                               bass.ts(nn, VW)], ev[:])

                k_proj_cc(0)
                v_proj_cc(0)
                ag("k", 0)
                ag("v", 0)
                # prefetch slab 0 (chunk 0, ranks 0+1) into outer tiles so
                # attention starts the moment the projections drain
                kt_pf = pf_p.tile([128, NCH * 2 * KCH], F32R, name="ktpf")
                for j in range(2):
                    for k in range(NCH):
                        nc.sync.dma_start(
                            kt_pf[:, bass.ds(k * 2 * KCH + j * KCH, KCH)],
                            k_gath[0][j][bass.ts(k, 128), :])
                v_pf = pf_p.tile([128, SLAB_KC * EMB], ATT, name="vpf")
                for j in range(2):
                    for kc in range(VKC):
                        nc.sync.dma_start(
                            v_pf[:, bass.ds((j * VKC + kc) * EMB, EMB)],
                            v_gath[0][j][bass.ts(kc, 128), :])
                k_proj_cc(1)
                v_proj_cc(1)

                wq_t = load_w(wqT, "a")  # reuse Wk slots
                for nn in range(QT_TILES):
                    pss = [psP_p.tile([128, QTW], F32, name=f"psq{m}",
                                      tag=f"pp{m}") for m in range(NCH)]
                    for k in range(NCH):
                        for m in range(NCH):
                            nc.tensor.matmul(pss[m][:],
                                             wq_t[k][:, bass.ts(m, 128)],
                                             xr[k][:, bass.ts(nn, QTW)],
                                             start=(k == 0), stop=(k == NCH - 1))
                    for m in range(NCH):
                        nc.scalar.activation(qt[m][:, bass.ts(nn, QTW)],
                                             pss[m][:],
                                             mybir.ActivationFunctionType.Identity,
                                             bias=bq_t[m][:])

            # Fence the proj->attention pool-slot handoff (sim-verified
            # DMA-queue under-sync on recycled slots). Remaining gathers are
            # issued after the fence so it does not wait on them.
            tc.strict_bb_all_engine_barrier()
            ag("k", 1)
            ag("v", 1)
            ag("k", 2)
            ag("v", 2)
            ag("k", 3)
            ag("v", 3)

            # ================= attention phase =================
            with (
                tc.tile_pool(name="kv", bufs=3) as kv_p,
                tc.tile_pool(name="pt", bufs=10) as pt_p,
                tc.tile_pool(name="fin", bufs=2) as fin_p,
                tc.tile_pool(name="psS", bufs=3, space="PSUM") as psS_p,
                tc.tile_pool(name="psO", bufs=3, space="PSUM") as psO_p,
                tc.tile_pool(name="psD", bufs=1, space="PSUM") as psD_p,
            ):
                d_ps = [psD_p.tile([128, QTW], F32, name=f"dps{q}",
                                   tag=f"d{q}") for q in range(QT_TILES)]

                # slabs: 512 keys = gather chunk c of ranks (2p, 2p+1)
                slabs = [(c, p) for c in range(G) for p in range(N_CORES // 2)]
                for si, (c, p) in enumerate(slabs):
                    first, last = si == 0, si == len(slabs) - 1
                    if first:
                        kt_slab, v_slab = kt_pf, v_pf
                    else:
                        kt_slab = kv_p.tile([128, NCH * 2 * KCH], F32R,
                                            name="kts", tag="kt")
                        for j in range(2):
                            for k in range(NCH):
                                nc.sync.dma_start(
                                    kt_slab[:, bass.ds(k * 2 * KCH + j * KCH,
                                                       KCH)],
                                    k_gath[c][2 * p + j][bass.ts(k, 128), :])
                        v_slab = kv_p.tile([128, SLAB_KC * EMB], ATT,
                                           name="vs", tag="vt")
                        for j in range(2):
                            for kc in range(VKC):
                                nc.sync.dma_start(
                                    v_slab[:, bass.ds((j * VKC + kc) * EMB,
                                                      EMB)],
                                    v_gath[c][2 * p + j]
                                    [bass.ts(kc, 128), :])

                    for q in range(QT_TILES):
                        pts = []
                        for kc in range(SLAB_KC):
                            ps = psS_p.tile([128, QTW], F32, name="pss",
                                            tag="ps")
                            for k in range(NCH):
                                nc.tensor.matmul(
                                    ps[:],
                                    kt_slab[:, bass.ds(k * 2 * KCH + kc * 128,
                                                       128)],
                                    qt[k][:, bass.ts(q, QTW)],
                                    start=(k == 0), stop=(k == NCH - 1))
                            pt = pt_p.tile([128, QTW], ATT, name="pt",
                                           tag="pt")
                            nc.scalar.activation(
                                pt[:], ps[:],
                                mybir.ActivationFunctionType.Exp, scale=SCALE)
                            pts.append(pt)
                        # presum the slab's P^T tiles on DVE (bf16 4x) so
                        # the denominator costs one PE matmul per slab
                        pp = pt_p.tile([128, QTW], ATT, name="pp", tag="pp",
                                       bufs=3)
                        nc.vector.tensor_add(pp[:], pts[0][:], pts[1][:])
                        for kc in range(2, SLAB_KC):
                            nc.vector.tensor_add(pp[:], pp[:], pts[kc][:])
                        nc.tensor.matmul(d_ps[q][:], ones_kr[:], pp[:],
                                         start=first, stop=last)
                        for m in range(NCH):
                            po = psO_p.tile([128, QTW], F32, name="pso",
                                            tag="po")
                            for kc in range(SLAB_KC):
                                nc.tensor.matmul(
                                    po[:],
                                    v_slab[:, bass.ds(kc * EMB + m * 128, 128)],
                                    pts[kc][:],
                                    start=(kc == 0), stop=(kc == SLAB_KC - 1))
                            if first:
                                nc.vector.tensor_copy(o_acc[q][m][:], po[:])
                            else:
                                nc.vector.tensor_add(o_acc[q][m][:],
                                                     o_acc[q][m][:], po[:])

                # ---- finalize: out^T = O^T * (1/D) + bv ----
                for q in range(QT_TILES):
                    bc = fin_p.tile([128, QTW], F32, name="bc", tag="bcs")
                    nc.vector.reciprocal(bc[:], d_ps[q][:])
                    for m in range(NCH):
                        fo = fin_p.tile([128, QTW], F32, name="fo", tag="fo",
                                        bufs=4)
                        nc.vector.tensor_mul(fo[:], o_acc[q][m][:], bc[:])
                        nc.vector.tensor_scalar_add(fo[:], fo[:], bv_t[m][:])
                        nc.sync.dma_start(
                            outT[bass.ts(m, 128), bass.ts(q, QTW)], fo[:])

    nc.compile()
    return nc


def _get_nc():
    if "nc" not in _COMPILED:
        _COMPILED["nc"] = _build()
    return _COMPILED["nc"]


def kernel(x, Wq, bq, Wk, bk, Wv, bv, _trace=False):
    _ensure_profile_hook()
    nc = _get_nc()
    x = np.ascontiguousarray(np.asarray(x, dtype=np.float32))
    in_maps = []
    wqT = np.ascontiguousarray(np.asarray(Wq, np.float32).T)
    wkT = np.ascontiguousarray(np.asarray(Wk, np.float32).T)
    wvT = np.ascontiguousarray(np.asarray(Wv, np.float32).T)
    bq = np.ascontiguousarray(np.asarray(bq, np.float32))
    bk = np.ascontiguousarray(np.asarray(bk, np.float32))
    bv = np.ascontiguousarray(np.asarray(bv, np.float32))
    for i in range(N_CORES):
        xT_i = np.ascontiguousarray(x[i * NQ:(i + 1) * NQ].T)
        in_maps.append({"xT": xT_i, "wqT": wqT, "wkT": wkT, "wvT": wvT,
                        "bq": bq, "bk": bk, "bv": bv})
    res = run_bass_kernel_spmd(nc, in_maps, core_ids=list(range(N_CORES)),
                               trace=_trace)
    out = np.concatenate([res.results[i]["outT"].T for i in range(N_CORES)],
                         axis=0)
    if _trace:
        return out, res
    return out
